# revision 8
# baseline (speedup 1.0000x reference)
"""ChebConv (K=5) Trainium2 Bass kernel, v2.

Problem: out = sum_k T_k(L) @ X @ W_k + bias, with L a random sparse (V,V)
matrix in COO form (E edges), X of shape (B, Cin, V) -> (V, B*Cin), Chebyshev
recurrence x_{k+1} = 2 L x_k - x_{k-1}.

Sharding: 8 cores = (batch b in 0..3) x (Cin half h in 0..1). Each core runs
the full Chebyshev recurrence on its 64-feature slice and produces a partial
(V, Cout) output contracted over its 64 Cin channels; the host sums the two
partials of each batch.

v2 changes vs baseline:
  - CHUNK=2048 (EBS=16) gathers; dynamic_dma_scratch_size=65536.
  - bf16 matmuls in the SpMM: S is a pure 0/1 one-hot built with a single
    is_equal (DVE); edge values are folded into the gathered payload on
    GPSIMD (tensor_tensor mult, fp32 -> bf16).
  - z tables kept fp32 node-major (gather needs 256B rows) plus a bf16
    feat-major z.T table written via PE transpose at z-update time; the
    final contraction reads contiguous [64, n*128] bf16 slices with no
    transposes. Phase 0 is gone: the host supplies x.T (z0) and x (z0.T).
  - z updates/stores batched into wide tiles (fewer DMA instructions).
"""

import numpy as np

# ---------------------------------------------------------------------------
# Problem constants (hardcoded per contest contract)
# ---------------------------------------------------------------------------
V = 50000
B = 4
CIN = 128
COUT = 128
K = 5
E = 800000
FC = 64                       # features per core (Cin half)
EBS = 8                       # edge-blocks per gather chunk
CHUNK = EBS * 128             # gather indices per dma_gather
N_CORES = 8
GRP = 8                       # dest blocks per wide store/load group

VP = ((V + 127) // 128) * 128        # 50048
NB = VP // 128                       # 391 dest blocks
HALF = VP // 2                       # 25024 (< int16 max)


# ---------------------------------------------------------------------------
# Host-side edge preprocessing (structure only: sort/pad/pack indices)
# ---------------------------------------------------------------------------
def _preprocess_edges(rows, cols, vals):
    """Sort edges by (source half, dest block), pad each (pass, db) group to a
    multiple of 128 edges and each pass to a multiple of EBS edge-blocks.

    Returns (idx_w, dlval, passes) where
      idx_w : (NCH, 128, CHUNK//16) int16, gather indices wrapped+replicated
      dlval : (NCH, 128, 2*EBS) f32, per-chunk dest-local and value columns
      passes: list over pass (lo/hi) of list of (db, n_ebs) in stream order
    """
    rows = np.asarray(rows).astype(np.int64)
    cols = np.asarray(cols).astype(np.int64)
    vals = np.asarray(vals).astype(np.float32)

    half = (cols >= HALF).astype(np.int64)
    db = rows // 128

    order = np.lexsort((rows, db, half))
    rows_s, cols_s, vals_s, half_s, db_s = (
        rows[order], cols[order], vals[order], half[order], db[order])

    idx_list, dl_list, val_list = [], [], []
    passes = []
    for p in (0, 1):
        sel = half_s == p
        r_p, c_p, v_p, db_p = rows_s[sel], cols_s[sel], vals_s[sel], db_s[sel]
        counts = np.bincount(db_p, minlength=NB)
        group_info = []
        off = 0
        p_idx, p_dl, p_val = [], [], []
        for d in range(NB):
            n = int(counts[d])
            gi = c_p[off:off + n] - p * HALF
            gd = (r_p[off:off + n] % 128).astype(np.float32)
            gv = v_p[off:off + n]
            off += n
            pad = (-n) % 128
            if n == 0:
                pad = 128  # ensure every (pass, db) group has >= 1 edge block
            if pad:
                gi = np.concatenate([gi, np.zeros(pad, np.int64)])
                gd = np.concatenate([gd, np.zeros(pad, np.float32)])
                gv = np.concatenate([gv, np.zeros(pad, np.float32)])
            p_idx.append(gi); p_dl.append(gd); p_val.append(gv)
            group_info.append((d, len(gi) // 128))
        # pad the pass stream to a whole number of chunks with dummy ebs
        # (attributed to the last dest block)
        tot_ebs = sum(g[1] for g in group_info)
        pad_ebs = (-tot_ebs) % EBS
        if pad_ebs:
            p_idx.append(np.zeros(pad_ebs * 128, np.int64))
            p_dl.append(np.zeros(pad_ebs * 128, np.float32))
            p_val.append(np.zeros(pad_ebs * 128, np.float32))
            d_last, n_last = group_info[-1]
            group_info[-1] = (d_last, n_last + pad_ebs)
        idx_list.append(np.concatenate(p_idx))
        dl_list.append(np.concatenate(p_dl))
        val_list.append(np.concatenate(p_val))
        passes.append(group_info)

    idx_all = np.concatenate(idx_list)
    dl_all = np.concatenate(dl_list)
    val_all = np.concatenate(val_list)
    n_edges = len(idx_all)
    assert n_edges % CHUNK == 0
    nch = n_edges // CHUNK

    assert idx_all.max() < 32768 and idx_all.min() >= 0

    # gather index wrapping: position i -> partition i%16, slot i//16,
    # replicated 8x across the 128 partitions.
    idx_w = idx_all.astype(np.int16).reshape(nch, CHUNK // 16, 16)
    idx_w = np.ascontiguousarray(idx_w.transpose(0, 2, 1))
    idx_w = np.ascontiguousarray(np.tile(idx_w, (1, 8, 1)))

    # per-chunk dest-local / val tiles: edge e of eb j -> row e%128, col j
    dl_c = dl_all.reshape(nch, EBS, 128).transpose(0, 2, 1)
    val_c = val_all.reshape(nch, EBS, 128).transpose(0, 2, 1)
    dlval = np.ascontiguousarray(
        np.concatenate([dl_c, val_c], axis=2).astype(np.float32))
    return idx_w, dlval, passes


# ---------------------------------------------------------------------------
# Bass program builder (identical for all 8 cores)
# ---------------------------------------------------------------------------
def _build_program(passes, nch, repeats=1):
    import concourse.bass as bass
    import concourse.bacc as bacc
    import concourse.mybir as mybir
    import concourse.tile as tile
    from concourse import library_config

    f32 = mybir.dt.float32
    bf16 = mybir.dt.bfloat16
    i16 = mybir.dt.int16
    AL = mybir.AluOpType

    nc = bacc.Bacc("TRN2", target_bir_lowering=False, debug=False,
                   num_swdge_queues=2, dynamic_dma_scratch_size=65536)

    z032 = nc.dram_tensor("z032", [VP, FC], f32, kind="ExternalInput")
    x64b = nc.dram_tensor("x64b", [FC, VP], bf16, kind="ExternalInput")
    wmat = nc.dram_tensor("wmat", [FC, K * COUT], bf16, kind="ExternalInput")
    biasr = nc.dram_tensor("biasr", [128, COUT], f32, kind="ExternalInput")
    iden = nc.dram_tensor("iden", [128, 128], f32, kind="ExternalInput")
    iotaf = nc.dram_tensor("iotaf", [128, 128], f32, kind="ExternalInput")
    idxs = nc.dram_tensor("idxs", [nch, 128, CHUNK // 16], i16,
                          kind="ExternalInput")
    dlval = nc.dram_tensor("dlval", [nch, 128, 2 * EBS], f32,
                           kind="ExternalInput")
    out = nc.dram_tensor("outp", [VP, COUT], f32, kind="ExternalOutput")

    # node-major fp32 z tables (gather source + zprev); z32[0] is the input
    z32 = [z032] + [nc.dram_tensor(f"z32_{k}", [VP, FC], f32, kind="Internal")
                    for k in range(1, K)]
    # feat-major bf16 z.T tables (final contraction); z16T[0] is the input
    z16T = [x64b] + [nc.dram_tensor(f"z16T_{k}", [FC, VP], bf16,
                                    kind="Internal")
                     for k in range(1, K)]

    with tile.TileContext(nc) as tc:
        nc.gpsimd.load_library(library_config.mlp)
        with (
            tc.tile_pool(name="const", bufs=1) as cpool,
            tc.tile_pool(name="part", bufs=1) as ppool,
            tc.tile_pool(name="idx", bufs=6) as ipool,
            tc.tile_pool(name="dv", bufs=6) as dpool,
            tc.tile_pool(name="gat", bufs=3) as gpool,
            tc.tile_pool(name="pay", bufs=3) as qpool,
            tc.tile_pool(name="sm", bufs=3) as spool,
            tc.tile_pool(name="zio", bufs=2) as ziop,
            tc.tile_pool(name="fin", bufs=2) as fpool,
            tc.tile_pool(name="psA", bufs=4, space="PSUM") as psumA,
            tc.tile_pool(name="psT", bufs=2, space="PSUM") as psumT,
            tc.tile_pool(name="psO", bufs=2, space="PSUM") as psumO,
        ):
            iden_t = cpool.tile([128, 128], f32, tag="iden")
            nc.sync.dma_start(iden_t[:], iden.ap())
            iota_t = cpool.tile([128, 128], f32, tag="iota")
            nc.sync.dma_start(iota_t[:], iotaf.ap())
            w_t = cpool.tile([FC, K * COUT], bf16, tag="w")
            nc.sync.dma_start(w_t[:], wmat.ap())
            bias_t = cpool.tile([128, COUT], f32, tag="bias")
            nc.sync.dma_start(bias_t[:], biasr.ap())
            part_t = ppool.tile([128, NB * FC], bf16, tag="part")

            for _rep in range(repeats):
                # ---- phases 1..K-1: Chebyshev SpMM steps ----
                gctr = 0            # global gather counter (queue parity)
                for k in range(1, K):
                    zin, zo32, zoT = z32[k - 1], z32[k], z16T[k]
                    scale = 1.0 if k == 1 else 2.0
                    ci = 0          # chunk cursor
                    jj = 0          # eb cursor within chunk
                    g_t = s_t = p_t = None
                    for p in (0, 1):
                        src = zin.ap()[p * HALF:(p + 1) * HALF, :]
                        # wide-tile state for pass 1 stores
                        zo_w = zT_w = None
                        g0 = -1      # first db of current store group
                        zp_w = None  # batched zprev loads (pass 0, k>=2)
                        zp0 = -1
                        for gi_idx, (d, n_ebs) in enumerate(passes[p]):
                            ps = psumA.tile([128, FC], f32, tag="pt")
                            for j in range(n_ebs):
                                if jj == 0:
                                    it = ipool.tile(
                                        [128, CHUNK // 16], i16, tag="idx")
                                    nc.sync.dma_start(it[:], idxs.ap()[ci])
                                    g_t = gpool.tile(
                                        [128, EBS, FC], f32, tag="g")
                                    nc.gpsimd.dma_gather(
                                        g_t[:], src, it[:],
                                        num_idxs=CHUNK, num_idxs_reg=CHUNK,
                                        elem_size=FC, queue_num=gctr % 2)
                                    dv_t = dpool.tile(
                                        [128, 2 * EBS], f32, tag="dv")
                                    nc.scalar.dma_start(
                                        dv_t[:], dlval.ap()[ci])
                                    dvv = dv_t[:].rearrange(
                                        "p (two e) -> p two e", two=2)
                                    # S[e,j,d] = (dl[e,j] == d)  (0/1 bf16)
                                    s_t = spool.tile(
                                        [128, EBS, 128], bf16, tag="s")
                                    nc.vector.tensor_tensor(
                                        s_t[:],
                                        iota_t[:].unsqueeze(1)
                                        .broadcast_to([128, EBS, 128]),
                                        dvv[:, 0, :].unsqueeze(2)
                                        .broadcast_to([128, EBS, 128]),
                                        AL.is_equal)
                                    # P[e,j,f] = val[e,j] * gathered[e,j,f]
                                    p_t = qpool.tile(
                                        [128, EBS, FC], bf16, tag="p")
                                    nc.gpsimd.tensor_tensor(
                                        p_t[:], g_t[:],
                                        dvv[:, 1, :].unsqueeze(2)
                                        .broadcast_to([128, EBS, FC]),
                                        AL.mult)
                                    gctr += 1
                                nc.tensor.matmul(
                                    ps[:], s_t[:, jj, :], p_t[:, jj, :],
                                    start=(j == 0), stop=(j == n_ebs - 1))
                                jj += 1
                                if jj == EBS:
                                    jj = 0
                                    ci += 1
                            pview = part_t[:, d * FC:(d + 1) * FC]
                            if p == 0:
                                if k == 1:
                                    # z1 = L z0: no zprev subtraction
                                    nc.vector.tensor_scalar_mul(
                                        pview, ps[:], scale)
                                else:
                                    if d - zp0 >= 4 or zp_w is None:
                                        zp_w = ziop.tile(
                                            [128, 4 * FC], f32, tag="zp")
                                        zp0 = d
                                        nzp = min(4, NB - d)
                                        nc.sync.dma_start(
                                            zp_w[:, :nzp * FC].rearrange(
                                                "p (n f) -> p n f", n=nzp),
                                            z32[k - 2].ap()
                                            [d * 128:(d + nzp) * 128, :]
                                            .rearrange("(n p) f -> p n f",
                                                       p=128))
                                    zpv = zp_w[:, (d - zp0) * FC:
                                               (d - zp0 + 1) * FC]
                                    nc.vector.scalar_tensor_tensor(
                                        pview, ps[:], scale, zpv,
                                        AL.mult, AL.subtract)
                            else:
                                if zo_w is None:
                                    g0 = d
                                    ng = min(GRP, NB - g0)
                                    zo_w = ziop.tile(
                                        [128, GRP * FC], f32, tag="zow")
                                    zT_w = ziop.tile(
                                        [FC, GRP * 128], bf16, tag="ztw")
                                i = d - g0
                                zov = zo_w[:, i * FC:(i + 1) * FC]
                                nc.vector.scalar_tensor_tensor(
                                    zov, ps[:], scale, pview,
                                    AL.mult, AL.add)
                                # transposed copy for the final contraction
                                pt = psumT.tile([FC, 128], f32, tag="tp")
                                nc.tensor.transpose(pt[:], zov, iden_t[:])
                                nc.vector.tensor_copy(
                                    zT_w[:, i * 128:(i + 1) * 128], pt[:])
                                if i == ng - 1:
                                    nc.sync.dma_start(
                                        zo32.ap()[g0 * 128:(g0 + ng) * 128, :]
                                        .rearrange("(n p) f -> p n f",
                                                   p=128),
                                        zo_w[:, :ng * FC].rearrange(
                                            "p (n f) -> p n f", n=ng))
                                    nc.scalar.dma_start(
                                        zoT.ap()[:, g0 * 128:(g0 + ng) * 128],
                                        zT_w[:, :ng * 128])
                                    zo_w = zT_w = None
                    assert jj == 0 and ci == nch

                # ---- final phase: out[db] = sum_k z_k.T @ W_k + bias ----
                for g0 in range(0, NB, GRP):
                    ng = min(GRP, NB - g0)
                    lhs_w = []
                    for k in range(K):
                        zw = fpool.tile([FC, GRP * 128], bf16, tag=f"zw{k}")
                        nc.sync.dma_start(
                            zw[:, :ng * 128],
                            z16T[k].ap()[:, g0 * 128:(g0 + ng) * 128])
                        lhs_w.append(zw)
                    ow = fpool.tile([128, GRP * COUT], f32, tag="ow")
                    for i in range(ng):
                        po = psumO.tile([128, COUT], f32, tag="po")
                        for k in range(K):
                            nc.tensor.matmul(
                                po[:], lhs_w[k][:, i * 128:(i + 1) * 128],
                                w_t[:, k * COUT:(k + 1) * COUT],
                                start=(k == 0), stop=(k == K - 1))
                        nc.vector.tensor_tensor(
                            ow[:, i * COUT:(i + 1) * COUT], po[:],
                            bias_t[:], AL.add)
                    nc.sync.dma_start(
                        out.ap()[g0 * 128:(g0 + ng) * 128, :]
                        .rearrange("(n p) f -> p n f", p=128),
                        ow[:, :ng * COUT].rearrange(
                            "p (n f) -> p n f", n=ng))

    nc.compile()
    return nc


# ---------------------------------------------------------------------------
# Host wrapper
# ---------------------------------------------------------------------------
_CACHE = {}


def build_in_maps(x, weight, bias, idx_w, dlval):
    import ml_dtypes
    bf16 = ml_dtypes.bfloat16
    x = np.asarray(x, np.float32)
    weight = np.asarray(weight, np.float32)
    bias = np.asarray(bias, np.float32)
    iden = np.eye(128, dtype=np.float32)
    iotaf = np.tile(np.arange(128, dtype=np.float32)[None, :], (128, 1))
    in_maps = []
    for c in range(N_CORES):
        b, h = c // 2, c % 2
        x_sl = x[b, h * FC:(h + 1) * FC, :]            # [FC, V]
        z032 = np.zeros((VP, FC), np.float32)
        z032[:V, :] = x_sl.T
        x64b = np.zeros((FC, VP), bf16)
        x64b[:, :V] = x_sl.astype(bf16)
        w_slice = np.ascontiguousarray(
            weight[:, h * FC:(h + 1) * FC, :].transpose(1, 0, 2)
        ).reshape(FC, K * COUT).astype(bf16)
        bias_r = np.tile(
            (bias if h == 0 else np.zeros_like(bias))[None, :], (128, 1)
        ).astype(np.float32)
        in_maps.append({
            "z032": z032, "x64b": x64b, "wmat": w_slice, "biasr": bias_r,
            "iden": iden, "iotaf": iotaf, "idxs": idx_w, "dlval": dlval,
        })
    return in_maps


def _get_runner(rows, cols, vals, repeats=1):
    key = ("prog", repeats)
    if key not in _CACHE:
        idx_w, dlval, passes = _preprocess_edges(rows, cols, vals)
        nch = idx_w.shape[0]
        nc = _build_program(passes, nch, repeats=repeats)
        _CACHE[key] = (nc, idx_w, dlval)
    return _CACHE[key]


def _run_spmd(nc, in_maps):
    from concourse.bass_utils import run_bass_kernel_spmd
    res = run_bass_kernel_spmd(nc, in_maps, core_ids=list(range(N_CORES)))
    return res.results


def kernel(x, lap_vals, weight, bias, lap_rows, lap_cols):
    import sys
    if '/opt/trn_rl_repo' not in sys.path:
        sys.path.insert(0, '/opt/trn_rl_repo')

    x = np.asarray(x, np.float32)
    lap_vals = np.asarray(lap_vals, np.float32)
    weight = np.asarray(weight, np.float32)
    bias = np.asarray(bias, np.float32)
    rows = np.asarray(lap_rows)
    cols = np.asarray(lap_cols)

    nc, idx_w, dlval = _get_runner(rows, cols, lap_vals)
    in_maps = build_in_maps(x, weight, bias, idx_w, dlval)
    results = _run_spmd(nc, in_maps)

    outf = np.empty((B, COUT, V), np.float32)
    for b in range(B):
        o = results[2 * b]["outp"] + results[2 * b + 1]["outp"]
        outf[b] = o[:V, :].T
    return outf


# revision 12
# speedup vs baseline: 5.3554x; 5.3554x over previous
"""ChebConv (K=5) Trainium2 Bass kernel, v2.

Problem: out = sum_k T_k(L) @ X @ W_k + bias, with L a random sparse (V,V)
matrix in COO form (E edges), X of shape (B, Cin, V) -> (V, B*Cin), Chebyshev
recurrence x_{k+1} = 2 L x_k - x_{k-1}.

Sharding: 8 cores = (batch b in 0..3) x (Cin half h in 0..1). Each core runs
the full Chebyshev recurrence on its 64-feature slice and produces a partial
(V, Cout) output contracted over its 64 Cin channels; the host sums the two
partials of each batch.

v2 changes vs baseline:
  - CHUNK=2048 (EBS=16) gathers; dynamic_dma_scratch_size=65536.
  - bf16 matmuls in the SpMM: S is a pure 0/1 one-hot built with a single
    is_equal (DVE); edge values are folded into the gathered payload on
    GPSIMD (tensor_tensor mult, fp32 -> bf16).
  - z tables kept fp32 node-major (gather needs 256B rows) plus a bf16
    feat-major z.T table written via PE transpose at z-update time; the
    final contraction reads contiguous [64, n*128] bf16 slices with no
    transposes. Phase 0 is gone: the host supplies x.T (z0) and x (z0.T).
  - z updates/stores batched into wide tiles (fewer DMA instructions).
"""

import numpy as np

# ---------------------------------------------------------------------------
# Problem constants (hardcoded per contest contract)
# ---------------------------------------------------------------------------
V = 50000
B = 4
CIN = 128
COUT = 128
K = 5
E = 800000
FC = 64                       # features per core (Cin half)
EBS = 8                       # edge-blocks per gather chunk
CHUNK = EBS * 128             # gather indices per dma_gather
N_CORES = 8
GRP = 8                       # dest blocks per wide store/load group

VP = ((V + 127) // 128) * 128        # 50048
NB = VP // 128                       # 391 dest blocks
HALF = VP // 2                       # 25024 (< int16 max)


# ---------------------------------------------------------------------------
# Host-side edge preprocessing (structure only: sort/pad/pack indices)
# ---------------------------------------------------------------------------
def _preprocess_edges(rows, cols, vals):
    """Sort edges by (source half, dest block), pad each (pass, db) group to a
    multiple of 128 edges and each pass to a multiple of EBS edge-blocks.

    Returns (idx_w, dlval, passes) where
      idx_w : (NCH, 128, CHUNK//16) int16, gather indices wrapped+replicated
      dlval : (NCH, 128, 2*EBS) f32, per-chunk dest-local and value columns
      passes: list over pass (lo/hi) of list of (db, n_ebs) in stream order
    """
    rows = np.asarray(rows).astype(np.int64)
    cols = np.asarray(cols).astype(np.int64)
    vals = np.asarray(vals).astype(np.float32)

    half = (cols >= HALF).astype(np.int64)
    db = rows // 128

    order = np.lexsort((rows, db, half))
    rows_s, cols_s, vals_s, half_s, db_s = (
        rows[order], cols[order], vals[order], half[order], db[order])

    idx_list, dl_list, val_list = [], [], []
    passes = []
    for p in (0, 1):
        sel = half_s == p
        r_p, c_p, v_p, db_p = rows_s[sel], cols_s[sel], vals_s[sel], db_s[sel]
        counts = np.bincount(db_p, minlength=NB)
        group_info = []
        off = 0
        p_idx, p_dl, p_val = [], [], []
        for d in range(NB):
            n = int(counts[d])
            gi = c_p[off:off + n] - p * HALF
            gd = (r_p[off:off + n] % 128).astype(np.float32)
            gv = v_p[off:off + n]
            off += n
            pad = (-n) % 128
            if n == 0:
                pad = 128  # ensure every (pass, db) group has >= 1 edge block
            if pad:
                gi = np.concatenate([gi, np.zeros(pad, np.int64)])
                gd = np.concatenate([gd, np.zeros(pad, np.float32)])
                gv = np.concatenate([gv, np.zeros(pad, np.float32)])
            p_idx.append(gi); p_dl.append(gd); p_val.append(gv)
            group_info.append((d, len(gi) // 128))
        # pad the pass stream to a whole number of chunks with dummy ebs
        # (attributed to the last dest block)
        tot_ebs = sum(g[1] for g in group_info)
        pad_ebs = (-tot_ebs) % EBS
        if pad_ebs:
            p_idx.append(np.zeros(pad_ebs * 128, np.int64))
            p_dl.append(np.zeros(pad_ebs * 128, np.float32))
            p_val.append(np.zeros(pad_ebs * 128, np.float32))
            d_last, n_last = group_info[-1]
            group_info[-1] = (d_last, n_last + pad_ebs)
        idx_list.append(np.concatenate(p_idx))
        dl_list.append(np.concatenate(p_dl))
        val_list.append(np.concatenate(p_val))
        passes.append(group_info)

    idx_all = np.concatenate(idx_list)
    dl_all = np.concatenate(dl_list)
    val_all = np.concatenate(val_list)
    n_edges = len(idx_all)
    assert n_edges % CHUNK == 0
    nch = n_edges // CHUNK

    assert idx_all.max() < 32768 and idx_all.min() >= 0

    # gather index wrapping: position i -> partition i%16, slot i//16,
    # replicated 8x across the 128 partitions.
    idx_w = idx_all.astype(np.int16).reshape(nch, CHUNK // 16, 16)
    idx_w = np.ascontiguousarray(idx_w.transpose(0, 2, 1))
    idx_w = np.ascontiguousarray(np.tile(idx_w, (1, 8, 1)))

    # per-chunk dest-local / val tiles: edge e of eb j -> row e%128, col j
    dl_c = dl_all.reshape(nch, EBS, 128).transpose(0, 2, 1)
    val_c = val_all.reshape(nch, EBS, 128).transpose(0, 2, 1)
    dlval = np.ascontiguousarray(
        np.concatenate([dl_c, val_c], axis=2).astype(np.float32))
    return idx_w, dlval, passes


# ---------------------------------------------------------------------------
# Bass program builder (identical for all 8 cores)
# ---------------------------------------------------------------------------
def _build_program(passes, nch, repeats=1, do_steps=True, do_final=True,
                   kmax=None):
    import concourse.bass as bass
    import concourse.bacc as bacc
    import concourse.mybir as mybir
    import concourse.tile as tile
    from concourse import library_config

    f32 = mybir.dt.float32
    bf16 = mybir.dt.bfloat16
    i16 = mybir.dt.int16
    AL = mybir.AluOpType

    nc = bacc.Bacc("TRN2", target_bir_lowering=False, debug=False,
                   num_swdge_queues=2, dynamic_dma_scratch_size=65536)

    z032 = nc.dram_tensor("z032", [VP, FC], f32, kind="ExternalInput")
    x64b = nc.dram_tensor("x64b", [FC, VP], bf16, kind="ExternalInput")
    wmat = nc.dram_tensor("wmat", [FC, K * COUT], bf16, kind="ExternalInput")
    biasr = nc.dram_tensor("biasr", [128, COUT], f32, kind="ExternalInput")
    iden = nc.dram_tensor("iden", [128, 128], f32, kind="ExternalInput")
    iotaf = nc.dram_tensor("iotaf", [128, 128], f32, kind="ExternalInput")
    idxs = nc.dram_tensor("idxs", [nch, 128, CHUNK // 16], i16,
                          kind="ExternalInput")
    dlval = nc.dram_tensor("dlval", [nch, 128, 2 * EBS], f32,
                           kind="ExternalInput")
    out = nc.dram_tensor("outp", [VP, COUT], f32, kind="ExternalOutput")

    # node-major fp32 z tables (gather source + zprev); z32[0] is the input
    z32 = [z032] + [nc.dram_tensor(f"z32_{k}", [VP, FC], f32, kind="Internal")
                    for k in range(1, K)]
    # feat-major bf16 z.T tables (final contraction); z16T[0] is the input
    z16T = [x64b] + [nc.dram_tensor(f"z16T_{k}", [FC, VP], bf16,
                                    kind="Internal")
                     for k in range(1, K)]

    with tile.TileContext(nc) as tc:
        nc.gpsimd.load_library(library_config.mlp)
        with (
            tc.tile_pool(name="const", bufs=1) as cpool,
            tc.tile_pool(name="part", bufs=1) as ppool,
            tc.tile_pool(name="idx", bufs=6) as ipool,
            tc.tile_pool(name="dv", bufs=6) as dpool,
            tc.tile_pool(name="gat", bufs=3) as gpool,
            tc.tile_pool(name="pay", bufs=3) as qpool,
            tc.tile_pool(name="sm", bufs=3) as spool,
            tc.tile_pool(name="zio", bufs=2) as ziop,
            tc.tile_pool(name="fin", bufs=2) as fpool,
            tc.tile_pool(name="psA", bufs=4, space="PSUM") as psumA,
            tc.tile_pool(name="psT", bufs=2, space="PSUM") as psumT,
            tc.tile_pool(name="psO", bufs=2, space="PSUM") as psumO,
        ):
            iden_t = cpool.tile([128, 128], f32, tag="iden")
            nc.sync.dma_start(iden_t[:], iden.ap())
            iota_t = cpool.tile([128, 128], f32, tag="iota")
            nc.sync.dma_start(iota_t[:], iotaf.ap())
            w_t = cpool.tile([FC, K * COUT], bf16, tag="w")
            nc.sync.dma_start(w_t[:], wmat.ap())
            bias_t = cpool.tile([128, COUT], f32, tag="bias")
            nc.sync.dma_start(bias_t[:], biasr.ap())
            part_t = ppool.tile([128, NB * FC], bf16, tag="part")

            for _rep in range(repeats):
                # ---- phases 1..K-1: Chebyshev SpMM steps ----
                gctr = 0            # global gather counter (queue parity)
                for k in range(1, (kmax or K) if do_steps else 1):
                    zin, zo32, zoT = z32[k - 1], z32[k], z16T[k]
                    scale = 1.0 if k == 1 else 2.0
                    ci = 0          # chunk cursor
                    jj = 0          # eb cursor within chunk
                    g_t = s_t = p_t = None
                    for p in (0, 1):
                        src = zin.ap()[p * HALF:(p + 1) * HALF, :]
                        # wide-tile state for pass 1 stores
                        zo_w = zT_w = None
                        g0 = -1      # first db of current store group
                        zp_w = None  # batched zprev loads (pass 0, k>=2)
                        zp0 = -1
                        for gi_idx, (d, n_ebs) in enumerate(passes[p]):
                            ps = psumA.tile([128, FC], f32, tag="pt")
                            for j in range(n_ebs):
                                if jj == 0:
                                    it = ipool.tile(
                                        [128, CHUNK // 16], i16, tag="idx")
                                    nc.sync.dma_start(it[:], idxs.ap()[ci])
                                    g_t = gpool.tile(
                                        [128, EBS, FC], f32, tag="g")
                                    nc.gpsimd.dma_gather(
                                        g_t[:], src, it[:],
                                        num_idxs=CHUNK, num_idxs_reg=CHUNK,
                                        elem_size=FC, queue_num=gctr % 2)
                                    dv_t = dpool.tile(
                                        [128, 2 * EBS], f32, tag="dv")
                                    nc.scalar.dma_start(
                                        dv_t[:], dlval.ap()[ci])
                                    dvv = dv_t[:].rearrange(
                                        "p (two e) -> p two e", two=2)
                                    # S[e,j,d] = (dl[e,j] == d)  (0/1 bf16)
                                    s_t = spool.tile(
                                        [128, EBS, 128], bf16, tag="s")
                                    nc.vector.tensor_tensor(
                                        s_t[:],
                                        iota_t[:].unsqueeze(1)
                                        .broadcast_to([128, EBS, 128]),
                                        dvv[:, 0, :].unsqueeze(2)
                                        .broadcast_to([128, EBS, 128]),
                                        AL.is_equal)
                                    # P[e,j,f] = val[e,j] * gathered[e,j,f]
                                    p_t = qpool.tile(
                                        [128, EBS, FC], bf16, tag="p")
                                    nc.vector.tensor_tensor(
                                        p_t[:], g_t[:],
                                        dvv[:, 1, :].unsqueeze(2)
                                        .broadcast_to([128, EBS, FC]),
                                        AL.mult)
                                    gctr += 1
                                nc.tensor.matmul(
                                    ps[:], s_t[:, jj, :], p_t[:, jj, :],
                                    start=(j == 0), stop=(j == n_ebs - 1))
                                jj += 1
                                if jj == EBS:
                                    jj = 0
                                    ci += 1
                            pview = part_t[:, d * FC:(d + 1) * FC]
                            if p == 0:
                                if k == 1:
                                    # z1 = L z0: no zprev subtraction
                                    nc.vector.tensor_scalar_mul(
                                        pview, ps[:], scale)
                                else:
                                    if d - zp0 >= 4 or zp_w is None:
                                        zp_w = ziop.tile(
                                            [128, 4 * FC], f32, tag="zp")
                                        zp0 = d
                                        nzp = min(4, NB - d)
                                        nc.sync.dma_start(
                                            zp_w[:, :nzp * FC].rearrange(
                                                "p (n f) -> p n f", n=nzp),
                                            z32[k - 2].ap()
                                            [d * 128:(d + nzp) * 128, :]
                                            .rearrange("(n p) f -> p n f",
                                                       p=128))
                                    zpv = zp_w[:, (d - zp0) * FC:
                                               (d - zp0 + 1) * FC]
                                    nc.vector.scalar_tensor_tensor(
                                        pview, ps[:], scale, zpv,
                                        AL.mult, AL.subtract)
                            else:
                                if zo_w is None:
                                    g0 = d
                                    ng = min(GRP, NB - g0)
                                    zo_w = ziop.tile(
                                        [128, GRP * FC], f32, tag="zow")
                                    zT_w = ziop.tile(
                                        [FC, GRP * 128], bf16, tag="ztw")
                                i = d - g0
                                zov = zo_w[:, i * FC:(i + 1) * FC]
                                nc.vector.scalar_tensor_tensor(
                                    zov, ps[:], scale, pview,
                                    AL.mult, AL.add)
                                # transposed copy for the final contraction
                                pt = psumT.tile([FC, 128], f32, tag="tp")
                                nc.tensor.transpose(pt[:], zov, iden_t[:])
                                nc.vector.tensor_copy(
                                    zT_w[:, i * 128:(i + 1) * 128], pt[:])
                                if i == ng - 1:
                                    nc.sync.dma_start(
                                        zo32.ap()[g0 * 128:(g0 + ng) * 128, :]
                                        .rearrange("(n p) f -> p n f",
                                                   p=128),
                                        zo_w[:, :ng * FC].rearrange(
                                            "p (n f) -> p n f", n=ng))
                                    nc.scalar.dma_start(
                                        zoT.ap()[:, g0 * 128:(g0 + ng) * 128],
                                        zT_w[:, :ng * 128])
                                    zo_w = zT_w = None
                    assert jj == 0 and (ci == nch or kmax is not None)

                # ---- final phase: out[db] = sum_k z_k.T @ W_k + bias ----
                for g0 in range(0, NB, GRP) if do_final else []:
                    ng = min(GRP, NB - g0)
                    lhs_w = []
                    for k in range(K):
                        zw = fpool.tile([FC, GRP * 128], bf16, tag=f"zw{k}")
                        nc.sync.dma_start(
                            zw[:, :ng * 128],
                            z16T[k].ap()[:, g0 * 128:(g0 + ng) * 128])
                        lhs_w.append(zw)
                    ow = fpool.tile([128, GRP * COUT], f32, tag="ow")
                    for i in range(ng):
                        po = psumO.tile([128, COUT], f32, tag="po")
                        for k in range(K):
                            nc.tensor.matmul(
                                po[:], lhs_w[k][:, i * 128:(i + 1) * 128],
                                w_t[:, k * COUT:(k + 1) * COUT],
                                start=(k == 0), stop=(k == K - 1))
                        nc.vector.tensor_tensor(
                            ow[:, i * COUT:(i + 1) * COUT], po[:],
                            bias_t[:], AL.add)
                    nc.sync.dma_start(
                        out.ap()[g0 * 128:(g0 + ng) * 128, :]
                        .rearrange("(n p) f -> p n f", p=128),
                        ow[:, :ng * COUT].rearrange(
                            "p (n f) -> p n f", n=ng))

    nc.compile()
    return nc


# ---------------------------------------------------------------------------
# Host wrapper
# ---------------------------------------------------------------------------
_CACHE = {}


def build_in_maps(x, weight, bias, idx_w, dlval):
    import ml_dtypes
    bf16 = ml_dtypes.bfloat16
    x = np.asarray(x, np.float32)
    weight = np.asarray(weight, np.float32)
    bias = np.asarray(bias, np.float32)
    iden = np.eye(128, dtype=np.float32)
    iotaf = np.tile(np.arange(128, dtype=np.float32)[None, :], (128, 1))
    in_maps = []
    for c in range(N_CORES):
        b, h = c // 2, c % 2
        x_sl = x[b, h * FC:(h + 1) * FC, :]            # [FC, V]
        z032 = np.zeros((VP, FC), np.float32)
        z032[:V, :] = x_sl.T
        x64b = np.zeros((FC, VP), bf16)
        x64b[:, :V] = x_sl.astype(bf16)
        w_slice = np.ascontiguousarray(
            weight[:, h * FC:(h + 1) * FC, :].transpose(1, 0, 2)
        ).reshape(FC, K * COUT).astype(bf16)
        bias_r = np.tile(
            (bias if h == 0 else np.zeros_like(bias))[None, :], (128, 1)
        ).astype(np.float32)
        in_maps.append({
            "z032": z032, "x64b": x64b, "wmat": w_slice, "biasr": bias_r,
            "iden": iden, "iotaf": iotaf, "idxs": idx_w, "dlval": dlval,
        })
    return in_maps


def _get_runner(rows, cols, vals, repeats=1):
    key = ("prog", repeats)
    if key not in _CACHE:
        idx_w, dlval, passes = _preprocess_edges(rows, cols, vals)
        nch = idx_w.shape[0]
        nc = _build_program(passes, nch, repeats=repeats)
        _CACHE[key] = (nc, idx_w, dlval)
    return _CACHE[key]


def _run_spmd(nc, in_maps):
    from concourse.bass_utils import run_bass_kernel_spmd
    res = run_bass_kernel_spmd(nc, in_maps, core_ids=list(range(N_CORES)))
    return res.results


def kernel(x, lap_vals, weight, bias, lap_rows, lap_cols):
    import sys
    if '/opt/trn_rl_repo' not in sys.path:
        sys.path.insert(0, '/opt/trn_rl_repo')

    x = np.asarray(x, np.float32)
    lap_vals = np.asarray(lap_vals, np.float32)
    weight = np.asarray(weight, np.float32)
    bias = np.asarray(bias, np.float32)
    rows = np.asarray(lap_rows)
    cols = np.asarray(lap_cols)

    nc, idx_w, dlval = _get_runner(rows, cols, lap_vals)
    in_maps = build_in_maps(x, weight, bias, idx_w, dlval)
    results = _run_spmd(nc, in_maps)

    outf = np.empty((B, COUT, V), np.float32)
    for b in range(B):
        o = results[2 * b]["outp"] + results[2 * b + 1]["outp"]
        outf[b] = o[:V, :].T
    return outf
